# revision 50
# baseline (speedup 1.0000x reference)
"""Batch-Jacobian of a 3-layer tanh MLP (64->256->256->64), B=8192.

J[b] = W3^T diag(1-h2^2) W2^T diag(1-h1^2) W1^T   (shape 64x64 per b)

U-path strategy (per core, 1024 batch elems, windows of 512):
  forward -> d1[k,b], d2[m,b] (bf16); window 1's forward is deferred into the
  pair loop so it overlaps window 0's jacobian work.
  Precompute U_i[k,m] = W1[i,k]*W2[k,m] host-side (8 MB bf16, stationary),
  streamed in i-chunks interleaved across the 4 (kh,mh) tensors; small
  constants are host-packed (xT|W1, W2, W3, b1|b2) so the first pair's
  operands land ~4us in.
  Pair loop (32 pairs x 4 i-values: even half -> psum partitions 0-63, odd
  half -> 64-127 via matmul tile_position), software-pipelined so PE's C2
  matmuls of pair n+1 are queued before J matmuls of pair n:
    C2[m,b | i,mh] = sum_k U_i[k,m] d1[k,b]  (PE; moving operand is d1 itself)
    C3 = C2 * d2[m,b]                        (DVE psum-drain mult, no bcast)
    J[j,b | i]  = sum_mh W3h^T C3            (PE, W3 stationary, 128-part psum)
    one ACT drain per pair -> jbuf [128, (ii,b)] -> immediate per-pair DMA
  Output DRAM layout [128, 32, BS] = [(s,j), ii, b] so all 128 partitions
  carry DMA traffic; host reassembles to [b, j, i].
"""

import sys

sys.path.insert(0, "/opt/trn_rl_repo")

import numpy as np
import ml_dtypes
from contextlib import ExitStack

import concourse.bass as bass
import concourse.mybir as mybir
import concourse.tile as tile
from concourse import bacc
from concourse.bass_utils import run_bass_kernel_spmd

B, D, H = 8192, 64, 256
NCORES = 8
BS = B // NCORES  # 1024 batch per core
WN = 512  # batch window
NW = BS // WN  # 2 windows
NP = 16  # pairs per window (each pair covers 4 i-values)
HW2C = 256  # half-window size
UCH = 8  # u streaming chunks (8 i-values each)

BF = mybir.dt.bfloat16
F32 = mybir.dt.float32
MUL = mybir.AluOpType.mult
ADD = mybir.AluOpType.add
Tanh = mybir.ActivationFunctionType.Tanh
Square = mybir.ActivationFunctionType.Square
Copy = mybir.ActivationFunctionType.Copy

_CACHE = {}
TRACE = False


def _build():
    nc = bacc.Bacc("TRN2")
    xt_d = nc.dram_tensor("xt", [D, BS], BF, kind="ExternalInput")
    # xw0 = [xT window 0 | W1] packed so one DMA covers all of them
    xw0_d = nc.dram_tensor("xw0", [D, 2 * HW2C + H], BF, kind="ExternalInput")
    # wp = [W2 half 0 | W2 half 1 | W3 half 0 | W3 half 1]
    wp_d = nc.dram_tensor("wp", [128, 2 * H + 2 * D], BF, kind="ExternalInput")
    b12_d = nc.dram_tensor("b12", [128, 4], F32, kind="ExternalInput")
    # u{kh}{mh}[k, i*128+m] = W1[i, kh*128+k] * W2[kh*128+k, mh*128+m]
    u_d = [
        [
            nc.dram_tensor(f"u{kh}{mh}", [128, D * 128], BF, kind="ExternalInput")
            for mh in range(2)
        ]
        for kh in range(2)
    ]
    # output layout [s*64+j, ii, b] with i = 4*(ii//2) + 2*s + ii%2;
    # host reassembles to [b, j, i]
    jac_d = nc.dram_tensor("jac", [128, D // 2, BS], F32, kind="ExternalOutput")

    with ExitStack() as ctx:
        tc = ctx.enter_context(tile.TileContext(nc))
        const = ctx.enter_context(tc.tile_pool(name="const", bufs=1))
        sb = ctx.enter_context(tc.tile_pool(name="sb", bufs=2))
        psf = ctx.enter_context(tc.tile_pool(name="psf", bufs=1, space="PSUM"))
        ps = ctx.enter_context(tc.tile_pool(name="ps", bufs=1, space="PSUM"))

        # ---- tiles ----
        xw0_sb = const.tile([D, 2 * HW2C + H], BF, name="xw0")
        w1_sb = xw0_sb[:, 2 * HW2C : 2 * HW2C + H]
        wp_sb = const.tile([128, 2 * H + 2 * D], BF, name="wp")
        w2_sb = [wp_sb[:, k * H : (k + 1) * H] for k in range(2)]
        w3_sb = [wp_sb[:, 2 * H + k * D : 2 * H + (k + 1) * D] for k in range(2)]
        b12_sb = const.tile([128, 4], F32, name="b12")
        b1_sb = b12_sb[:, 0:2]
        b2_sb = b12_sb[:, 2:4]
        xt1_sb = const.tile([D, WN], BF, name="xt1")
        u_sb = [
            [const.tile([128, D * 128], BF, name=f"u{kh}{mh}") for mh in range(2)]
            for kh in range(2)
        ]

        HW2 = WN // 2
        d1 = [[None, None] for _ in range(NW)]
        d2 = [[None, None] for _ in range(NW)]
        # per-half d1/d2 tiles (only pair 0 of window 0 reads these, so it can
        # start before the full-window forward completes)
        d1h = [[None, None], [None, None]]  # [half][kh]
        d2h = [[None, None], [None, None]]  # [half][mh]

        h1store = {}

        def fwd_l1(w, h):
            """Layer 1 for batch half h of window w: a1 -> tanh -> d1 chain
            (C2 needs d1 first). Writes half-slices of the full d1 tiles;
            for w==0 also writes standalone half tiles."""
            bsl = slice(h * HW2, (h + 1) * HW2)
            h1 = [
                sb.tile([128, HW2], BF, tag=f"h1h_{k}", name=f"h1h_{k}")
                for k in range(2)
            ]
            h1store[(w, h)] = h1
            for hh in range(2):
                a_ps = psf.tile([128, HW2], F32, tag="fwd", name="a1_ps", bufs=2)
                nc.tensor.matmul(
                    a_ps,
                    w1_sb[:, hh * 128 : (hh + 1) * 128],
                    xTh[w][h],
                    start=True,
                    stop=True,
                )
                nc.scalar.activation(
                    out=h1[hh], in_=a_ps, func=Tanh, bias=b1_sb[:, hh : hh + 1]
                )
            for hh in range(2):
                sq = sb.tile([128, HW2], BF, tag="sq", name="sq1")
                if w == 0:
                    nc.vector.tensor_tensor(out=sq, in0=h1[hh], in1=h1[hh], op=MUL)
                else:
                    # window 1 runs mid-body where DVE is the busier engine
                    nc.scalar.activation(out=sq, in_=h1[hh], func=Square)
                nc.vector.tensor_scalar(
                    out=d1[w][hh][:, bsl],
                    in0=sq, scalar1=-1.0, scalar2=1.0, op0=MUL, op1=ADD,
                )
                if w == 0:
                    nc.vector.tensor_scalar(
                        out=d1h[h][hh],
                        in0=sq, scalar1=-1.0, scalar2=1.0, op0=MUL, op1=ADD,
                    )

        def fwd_l2(w, h):
            """Layer 2 for batch half h of window w -> d2."""
            bsl = slice(h * HW2, (h + 1) * HW2)
            h1 = h1store.pop((w, h))
            for mh in range(2):
                a_ps = psf.tile([128, HW2], F32, tag="fwd", name="a2_ps", bufs=2)
                for hh in range(2):
                    nc.tensor.matmul(
                        a_ps,
                        w2_sb[hh][:, mh * 128 : (mh + 1) * 128],
                        h1[hh],
                        start=(hh == 0),
                        stop=(hh == 1),
                    )
                h2 = sb.tile([128, HW2], BF, tag="h2", name="h2")
                nc.scalar.activation(
                    out=h2, in_=a_ps, func=Tanh, bias=b2_sb[:, mh : mh + 1]
                )
                sq = sb.tile([128, HW2], BF, tag="sq", name="sq2")
                if w == 0:
                    nc.vector.tensor_tensor(out=sq, in0=h2, in1=h2, op=MUL)
                else:
                    nc.scalar.activation(out=sq, in_=h2, func=Square)
                nc.vector.tensor_scalar(
                    out=d2[w][mh][:, bsl],
                    in0=sq, scalar1=-1.0, scalar2=1.0, op0=MUL, op1=ADD,
                )
                if w == 0:
                    nc.vector.tensor_scalar(
                        out=d2h[h][mh],
                        in0=sq, scalar1=-1.0, scalar2=1.0, op0=MUL, op1=ADD,
                    )

        for w in range(NW):
            for kk in range(2):
                d1[w][kk] = sb.tile(
                    [128, WN], BF, tag=f"d1_{w}_{kk}", name=f"d1_{w}_{kk}", bufs=1
                )
                d2[w][kk] = sb.tile(
                    [128, WN], BF, tag=f"d2_{w}_{kk}", name=f"d2_{w}_{kk}", bufs=1
                )
        for h in range(2):
            for kk in range(2):
                d1h[h][kk] = sb.tile(
                    [128, HW2], BF, tag=f"d1h_{h}_{kk}", name=f"d1h_{h}_{kk}", bufs=1
                )
                d2h[h][kk] = sb.tile(
                    [128, HW2], BF, tag=f"d2h_{h}_{kk}", name=f"d2h_{h}_{kk}", bufs=1
                )

        # ---- DMA order: tuned so pair 0's operands (d1 half 0 and the
        # first 4-i u slices) arrive as early as possible; small consts are
        # host-packed so each 500ns DMA slot carries more ----
        xTh = [
            [xw0_sb[:, h * HW2 : (h + 1) * HW2] for h in range(2)],
            [xt1_sb[:, h * HW2 : (h + 1) * HW2] for h in range(2)],
        ]

        # preload the ACT function table (1.3us) off the critical path
        warm = sb.tile([1, 2], F32, tag="warm", name="warm", bufs=1)
        nc.gpsimd.memset(warm, 0.0)
        nc.scalar.activation(out=warm[:, 0:1], in_=warm[:, 1:2], func=Tanh)

        usl0 = slice(0, 4 * 128)
        nc.sync.dma_start(out=xw0_sb, in_=xw0_d[:, :])
        nc.sync.dma_start(out=b12_sb, in_=b12_d[:, :])
        nc.sync.dma_start(out=wp_sb, in_=wp_d[:, :])
        for kh in range(2):
            nc.sync.dma_start(out=u_sb[kh][0][:, usl0], in_=u_d[kh][0][:, usl0])
        for kh in range(2):
            nc.sync.dma_start(out=u_sb[kh][1][:, usl0], in_=u_d[kh][1][:, usl0])
        nc.sync.dma_start(out=xt1_sb, in_=xt_d[:, WN : 2 * WN])
        # remaining u chunks (first 4 i-values already in flight above)
        uch = [4] + [8] * 7
        c0 = 4
        for cw in uch:
            sl = slice(c0 * 128, (c0 + cw) * 128)
            for mh in range(2):
                for kh in range(2):
                    nc.sync.dma_start(out=u_sb[kh][mh][:, sl], in_=u_d[kh][mh][:, sl])
            c0 += cw

        fwd_l1(0, 0)
        fwd_l1(0, 1)
        fwd_l2(0, 0)
        fwd_l2(0, 1)

        # ---- pipelined pair loop ----
        TOT = NW * NP

        QS = tuple((qq * WN // 4, (qq + 1) * WN // 4) for qq in range(4))

        def emit_c2(n, split=False, halftiles=False):
            w, t = divmod(n, NP)
            c3 = [[None, None], [None, None]]  # [s][mh]
            halves = (
                ((0, WN),) if not split
                else (QS if n == TOT - 1 else ((0, HW2), (HW2, WN)))
            )
            for s in range(2):
                for mh in range(2):
                    c2_ps = ps.tile([128, 2 * WN], F32, tag="c2", name="c2_ps", bufs=2)
                    ct = sb.tile(
                        [128, 2 * WN], BF, tag=f"c3_{s}_{mh}", name=f"c3_{s}_{mh}"
                    )
                    for hi, (b0, b1) in enumerate(halves):
                        if halftiles:
                            rhs1 = [d1h[hi][kh] for kh in range(2)]
                            mul2 = d2h[hi][mh]
                        else:
                            rhs1 = [d1[w][kh][:, b0:b1] for kh in range(2)]
                            mul2 = d2[w][mh][:, b0:b1]
                        for q in range(2):
                            i = 4 * t + 2 * s + q
                            for kh in range(2):
                                nc.tensor.matmul(
                                    c2_ps[:, q * WN + b0 : q * WN + b1],
                                    u_sb[kh][mh][:, i * 128 : (i + 1) * 128],
                                    rhs1[kh],
                                    start=(kh == 0),
                                    stop=(kh == 1),
                                )
                        nc.vector.tensor_tensor(
                            out=ct.rearrange("p (q b) -> p q b", q=2)[:, :, b0:b1],
                            in0=c2_ps.rearrange("p (q b) -> p q b", q=2)[:, :, b0:b1],
                            in1=mul2[:, None, :].broadcast_to([128, 2, b1 - b0]),
                            op=MUL,
                        )
                    c3[s][mh] = ct
            return c3

        def emit_j(n, c3, split=False):
            w, t = divmod(n, NP)
            j_ps = ps.tile([128, 2 * WN], F32, tag="jps", name="j_ps", bufs=1)
            jb = sb.tile([128, 2 * WN], F32, tag="jb", name="jb", bufs=2)
            wb = w * WN
            halves = (
                ((0, WN),) if not split
                else (QS if n == TOT - 1 else ((0, HW2), (HW2, WN)))
            )
            for b0, b1 in halves:
                for s in range(2):
                    pview = j_ps[s * 64 : (s + 1) * 64, :]
                    for mh in range(2):
                        for q in range(2):
                            nc.tensor.matmul(
                                pview[:, q * WN + b0 : q * WN + b1],
                                w3_sb[mh],
                                c3[s][mh][:, q * WN + b0 : q * WN + b1],
                                start=(mh == 0),
                                stop=(mh == 1),
                            )
                nc.scalar.activation(
                    out=jb.rearrange("p (q b) -> p q b", q=2)[:, :, b0:b1],
                    in_=j_ps.rearrange("p (q b) -> p q b", q=2)[:, :, b0:b1],
                    func=Copy,
                )
                nc.sync.dma_start(
                    out=jac_d[:, 2 * t : 2 * t + 2, wb + b0 : wb + b1],
                    in_=jb.rearrange("p (q b) -> p q b", q=2)[:, :, b0:b1],
                )

        prev = None
        for n in range(TOT):
            c3 = emit_c2(
                n, split=(n == 0 or n == TOT - 1), halftiles=(n == 0)
            )
            if n == 1:
                fwd_l1(1, 0)
                fwd_l1(1, 1)
            if n == 5:
                fwd_l2(1, 0)
            if n == 7:
                fwd_l2(1, 1)
            if prev is not None:
                emit_j(*prev)
            prev = (n, c3)
        emit_j(prev[0], prev[1], split=True)
    nc.compile()
    return nc


def kernel(x, W1, b1, W2, b2, W3, b3):
    x = np.asarray(x, dtype=np.float32)
    bf = ml_dtypes.bfloat16
    if "nc" not in _CACHE:
        _CACHE["nc"] = _build()
    nc = _CACHE["nc"]

    W1f = np.asarray(W1, np.float32)
    W2f = np.asarray(W2, np.float32)
    Ufull = W1f[:, :, None] * W2f[None, :, :]  # [i, k, m]
    b1f = np.asarray(b1, np.float32)
    b2f = np.asarray(b2, np.float32)
    W3f = np.asarray(W3, np.float32)
    shared = {
        "wp": np.concatenate(
            [W2f[0:128], W2f[128:256], W3f[0:128], W3f[128:256]], axis=1
        ).astype(bf),
        "b12": np.concatenate(
            [b1f.reshape(2, 128).T, b2f.reshape(2, 128).T], axis=1
        ).astype(np.float32),
    }
    for kh in range(2):
        for mh in range(2):
            u = Ufull[:, kh * 128 : (kh + 1) * 128, mh * 128 : (mh + 1) * 128]
            u = np.ascontiguousarray(u.transpose(1, 0, 2).reshape(128, D * 128))
            shared[f"u{kh}{mh}"] = u.astype(bf)
    xt = np.ascontiguousarray(x.T.astype(bf))  # [D, B]
    w1b = W1f.astype(bf)
    in_maps = [
        {
            "xt": np.ascontiguousarray(xt[:, c * BS : (c + 1) * BS]),
            "xw0": np.ascontiguousarray(
                np.concatenate([xt[:, c * BS : c * BS + 2 * HW2C], w1b], axis=1)
            ),
            **shared,
        }
        for c in range(NCORES)
    ]
    res = run_bass_kernel_spmd(nc, in_maps, core_ids=list(range(NCORES)), trace=TRACE)
    _CACHE["last_res"] = res
    out = np.empty((B, D, D), np.float32)
    for c in range(NCORES):
        # jac[s*64+j, 2t+q, b] = J[j, b, 4t+2s+q]
        arr = res.results[c]["jac"].reshape(2, 64, 16, 2, BS)
        out[c * BS : (c + 1) * BS] = (
            arr.transpose(4, 1, 2, 0, 3).reshape(BS, D, D)
        )
    return out
